# revision 25
# baseline (speedup 1.0000x reference)
"""EDRN cell kernel for Trainium2, data-parallel over batch across 8 NeuronCores.

Strategy:
  - Shard batch B=1024 into 8 slices of 128 rows; replicate weights (fp16).
  - Mapping: batch rows on PSUM partitions, gate columns on the free dim.
    Stationary operands are transposed activation slices (host-prepped fp16,
    laid out [128, c, 128] so DMAs are contiguous), moving operands are weight
    row-chunks (fp16), fp32 PSUM accumulation.
  - Per gate the B(x)-part and A-part weights are merged host-side into one
    [128, 6, 2048] array with the x-part first, streamed as 3 ascending-size
    DMAs; biases are K=1 matmuls against a ones-row stationary.
  - fg/in run k-outer (matmuls chase the weight stream); th/ot run n-outer so
    each 512-column chunk finishes (incl. activation) early and the a_new/aa
    elementwise phases pipeline chunk-by-chunk with the remaining matmuls.
  - ot's x-part runs as a separate early accumulation phase overlapping the
    a_new elementwise work; its a_new_last-part joins per column chunk.
  - A_pt (block-diagonal strictly-upper 4x4) -> 16 [128,128] diag-block
    matmuls; a_new_last / aa_last transposed on the PE (transpose mode).
Elementwise math is fp32; matmul operands and the streamed copy of `a` are
fp16 (measured end-to-end error ~4e-4 relative).
"""

import numpy as np

import concourse.bass as bass
import concourse.mybir as mybir
import concourse.tile as tile
from concourse import bacc
from concourse.bass_utils import run_bass_kernel_spmd

N, M, D = 256, 512, 4
MD = M * D  # 2048
B = 1024
NCORES = 8
BL = B // NCORES  # 128 batch rows per core

F16 = mybir.dt.float16
F32 = mybir.dt.float32

KM = M // 128    # 4 K-chunks of the m/a_last contraction
KX = N // 128    # 2 K-chunks of the x contraction
KW = KM + KX     # 6 merged weight K-chunks per gate (x-part first)
NCH = MD // 512  # 4 column chunks of 512
SEGS = [(k, 1) for k in range(6)]  # single-chunk weight DMAs: pace matmuls per 512KB

# stat_a1: a_last^T chunks (tiny, lands between th's x and A weight chunks)
SA1_F = 512
# stat_a2 blob: a^T | A_pt diag blocks | a (natural fp16) | identity
SA_AT = 0
SA_APT = 2048
SA_A16 = 4096
SA_ID = 6144
SA2_F = SA_ID + 128  # 6272
# stat_b blob: A_st chunks only (needed last, keeps the ring tail short)
SB_F = 2048

_CACHE = {}
LAST_RESULT = None  # BassKernelResults of the most recent run (for test harness)


def _build():
    nc = bacc.Bacc(
        "TRN2", target_bir_lowering=False, debug=False, num_devices=NCORES
    )

    def din(name, shape, dt):
        return nc.dram_tensor(name, shape, dt, kind="ExternalInput").ap()

    def dout(name, shape, dt):
        return nc.dram_tensor(name, shape, dt, kind="ExternalOutput").ap()

    mxT = din("mxT16", [128, KW, 128], F16)   # x^T chunks 0..1, m^T chunks 2..5
    stat_a1 = din("stat_a1_16", [128, SA1_F], F16)
    stat_a2 = din("stat_a2_16", [128, SA2_F], F16)
    stat_b = din("stat_b16", [128, SB_F], F16)
    wball = din("wball16", [1, 4 * MD + 128], F16)  # 4 bias rows + ones row
    Wg = {g: din(f"W_{g}", [128, KW, MD], F16) for g in ("fg", "in", "th", "ot")}
    m_out = dout("m_new_out", [BL, M], F32)
    a_out = dout("a_new_out", [BL, MD], F32)

    GIDX = {"fg": 0, "in": 1, "th": 2, "ot": 3}
    AF = mybir.ActivationFunctionType

    with tile.TileContext(nc) as tc:
        with (
            tc.tile_pool(name="singles", bufs=1) as singles,
            tc.tile_pool(name="wpool", bufs=6) as wpool,
            tc.tile_pool(name="psum", bufs=8, space="PSUM") as pp,
            tc.tile_pool(name="work", bufs=3) as work,
        ):
            # mxT first on the sync ring (feeds the very first matmuls);
            # biases on the scalar ring (needed only at gate tails)
            smxT = singles.tile([128, KW, 128], F16, tag="smxT")
            nc.scalar.dma_start(out=smxT, in_=mxT)
            swb = singles.tile([1, 4 * MD + 128], F16, tag="swb")
            nc.scalar.dma_start(out=swb, in_=wball)
            sones = swb[:, 4 * MD : 4 * MD + 128]

            # PE warmup: dummy matmuls during the initial DMA prefix keep the
            # HAM activity window busy so the real stream starts at 2.4GHz
            # instead of the throttled 1.2GHz default.
            wu = singles.tile([128, 256], F16, tag="wu")
            nc.vector.memset(wu, 0.0)
            wups = pp.tile([64, 256], F32, tag="ps", name="wups")
            for _ in range(24):
                nc.tensor.matmul(
                    wups, lhsT=wu[:, 0:64], rhs=wu, start=True, stop=True
                )

            def lhs_for(statA, k):
                if k < KX or statA is None:
                    return smxT[:, k, :]
                return statA[:, k - KX, :]

            def gate_load(gname):
                tiles = []
                for start_c, nch in SEGS:
                    w = wpool.tile(
                        [128, nch, MD], F16, tag=f"w{nch}", name=f"w_{gname}_{start_c}"
                    )
                    nc.sync.dma_start(
                        out=w, in_=Wg[gname][:, start_c : start_c + nch, :]
                    )
                    for kk in range(nch):
                        tiles.append((w, kk))
                return tiles

            def bias_mm(gname, psums, n, stop=True):
                boff = GIDX[gname] * MD
                nc.tensor.matmul(
                    psums[n],
                    lhsT=sones,
                    rhs=swb[:, boff + 512 * n : boff + 512 * (n + 1)],
                    start=False,
                    stop=stop,
                )

            def new_psums(gname):
                return [
                    pp.tile([128, 512], F32, tag="ps", name=f"ps_{gname}_{n}")
                    for n in range(NCH)
                ]

            def gate_kcontig(gname, func):
                """k-outer: matmuls chase the weight stream (fg/in)."""
                G = singles.tile([128, MD], F32, tag=f"G_{gname}")
                psums = new_psums(gname)
                for k, (w, kk) in enumerate(gate_load(gname)):
                    lhsT = lhs_for(None, k)
                    for n in range(NCH):
                        nc.tensor.matmul(
                            psums[n],
                            lhsT=lhsT,
                            rhs=w[:, kk, 512 * n : 512 * (n + 1)],
                            start=(k == 0),
                            stop=False,
                        )
                for n in range(NCH):
                    bias_mm(gname, psums, n)
                    nc.scalar.activation(
                        out=G[:, 512 * n : 512 * (n + 1)], in_=psums[n], func=func
                    )
                return G

            G_fg = gate_kcontig("fg", AF.Sigmoid)

            # stationaries for th / a_new, loaded while fg/in stream
            ssa = singles.tile([128, SA_F], F16, tag="ssa")
            nc.gpsimd.dma_start(out=ssa, in_=stat_a)
            salastT = ssa[:, SA_ALAST : SA_ALAST + 512].rearrange(
                "p (c k) -> p c k", k=128
            )
            saT = ssa[:, SA_AT : SA_AT + 2048].rearrange("p (c k) -> p c k", k=128)
            sAptd = ssa[:, SA_APT : SA_APT + 2048].rearrange(
                "p (c k) -> p c k", k=128
            )
            sa16 = ssa[:, SA_A16 : SA_A16 + 2048]
            sident = ssa[:, SA_ID : SA_ID + 128]

            G_in = gate_kcontig("in", AF.Sigmoid)

            ssb = singles.tile([128, SB_F], F16, tag="ssb")
            nc.gpsimd.dma_start(out=ssb, in_=stat_b)
            sAst = ssb[:, SB_AST : SB_AST + 2048].rearrange(
                "p (c m) -> p c m", m=512
            )
            sa16 = ssb[:, SB_A16 : SB_A16 + 2048]
            sident = ssb[:, SB_ID : SB_ID + 128]

            def transpose128(src16, dst, n):
                """dst[:, n, :] = src16[:, 128n:128(n+1)].T via PE transpose."""
                pt = pp.tile([128, 128], F16, tag="ps", name=f"pt_{dst.name}_{n}")
                nc.tensor.transpose(
                    pt, src16[:, 128 * n : 128 * (n + 1)], sident
                )
                nc.vector.tensor_copy(dst[:, n, :], pt)

            # ---- th gate (k-outer), then a_new phase ----
            # ring order: th x-chunks, a_last^T (tiny), th A-chunks, then the
            # bulky a^T/A_pt/a/ident blob draining behind th's matmuls
            th_tiles = []

            def _load_th(lo, hi):
                for c in range(lo, hi):
                    w = wpool.tile([128, 1, MD], F16, tag="w1", name=f"w_th_{c}")
                    nc.sync.dma_start(out=w, in_=Wg["th"][:, c : c + 1, :])
                    th_tiles.append((w, 0))

            _load_th(0, KX)
            ssa1 = singles.tile([128, SA1_F], F16, tag="ssa1")
            nc.sync.dma_start(out=ssa1, in_=stat_a1)
            salastT = ssa1.rearrange("p (c k) -> p c k", k=128)
            _load_th(KX, KW)
            ssa2 = singles.tile([128, SA2_F], F16, tag="ssa2")
            nc.sync.dma_start(out=ssa2, in_=stat_a2)
            saT = ssa2[:, SA_AT : SA_AT + 2048].rearrange("p (c k) -> p c k", k=128)
            sAptd = ssa2[:, SA_APT : SA_APT + 2048].rearrange(
                "p (c k) -> p c k", k=128
            )
            sa16 = ssa2[:, SA_A16 : SA_A16 + 2048]
            sident = ssa2[:, SA_ID : SA_ID + 128]

            G_th = singles.tile([128, MD], F32, tag="G_th")
            th_psums = new_psums("th")
            for k, (w, kk) in enumerate(th_tiles):
                lhsT = lhs_for(salastT, k)
                for n in range(NCH):
                    nc.tensor.matmul(
                        th_psums[n],
                        lhsT=lhsT,
                        rhs=w[:, kk, 512 * n : 512 * (n + 1)],
                        start=(k == 0),
                        stop=False,
                    )
            for n in range(NCH):
                for s in range(4):
                    c = 4 * n + s
                    nc.tensor.matmul(
                        th_psums[n][:, 128 * s : 128 * (s + 1)],
                        lhsT=saT[:, c, :],
                        rhs=sAptd[:, c, :],
                        start=False,
                        stop=False,
                        skip_group_check=True,
                    )
            for n in range(NCH):
                bias_mm("th", th_psums, n)
                nc.scalar.activation(
                    out=G_th[:, 512 * n : 512 * (n + 1)],
                    in_=th_psums[n],
                    func=AF.Tanh,
                )

            # ---- ot x-part first: independent of a_new, fills the PE while
            #      the a_new elementwise phase runs ----
            G_ot = singles.tile([128, MD], F32, tag="G_ot")
            ot_psums = new_psums("ot")
            ot_tiles = gate_load("ot")
            ssb = singles.tile([128, SB_F], F16, tag="ssb")
            nc.sync.dma_start(out=ssb, in_=stat_b)
            sAst = ssb.rearrange("p (c m) -> p c m", m=512)
            for k, (w, kk) in enumerate(ot_tiles[:KX]):
                for n in range(NCH):
                    nc.tensor.matmul(
                        ot_psums[n],
                        lhsT=smxT[:, k, :],
                        rhs=w[:, kk, 512 * n : 512 * (n + 1)],
                        start=(k == 0),
                        stop=False,
                    )

            # ---- a_new = a * G_fg + G_th * G_in, plus last-slice transpose ----
            a_new = singles.tile([128, MD], F32, tag="a_new")
            anl16 = singles.tile([128, 512], F16, tag="anl16")
            sanlT = singles.tile([128, KM, 128], F16, tag="sanlT")
            for n in range(NCH):
                sl = slice(512 * n, 512 * (n + 1))
                t1 = work.tile([128, 512], F32, tag="t1")
                nc.vector.tensor_mul(t1, G_th[:, sl], G_in[:, sl])
                t2 = work.tile([128, 512], F32, tag="t2")
                nc.gpsimd.tensor_mul(t2, sa16[:, sl], G_fg[:, sl])
                nc.vector.tensor_add(a_new[:, sl], t1, t2)
                lastview = a_new[:, sl].rearrange("p (m s) -> p m s", s=4)[:, :, 3]
                nc.vector.tensor_copy(anl16[:, 128 * n : 128 * (n + 1)], lastview)
                transpose128(anl16, sanlT, n)
            nc.gpsimd.dma_start(out=a_out, in_=a_new)

            # ---- ot a_new_last-part ----
            for k, (w, kk) in list(enumerate(ot_tiles))[KX:]:
                for n in range(NCH):
                    nc.tensor.matmul(
                        ot_psums[n],
                        lhsT=sanlT[:, k - KX, :],
                        rhs=w[:, kk, 512 * n : 512 * (n + 1)],
                        start=False,
                        stop=False,
                    )
            # tanh(a_new) is independent of G_ot — compute it during the ot
            # matmul phase while the scalar queue is otherwise idle
            tanh_a = singles.tile([128, MD], F32, tag="tanh_a")
            for n in range(NCH):
                sl = slice(512 * n, 512 * (n + 1))
                nc.scalar.activation(
                    out=tanh_a[:, sl], in_=a_new[:, sl], func=AF.Tanh
                )

            for n in range(NCH):
                bias_mm("ot", ot_psums, n)
                nc.scalar.activation(
                    out=G_ot[:, 512 * n : 512 * (n + 1)],
                    in_=ot_psums[n],
                    func=AF.Sigmoid,
                )

            # ---- aa = tanh(a_new) * G_ot; m_new accumulates as chunks finish ----
            aa = singles.tile([128, MD], F32, tag="aa")
            aal16 = singles.tile([128, 512], F16, tag="aal16")
            saalT = singles.tile([128, KM, 128], F16, tag="saalT")
            s012 = singles.tile([128, 512], F32, tag="s012")
            psm = pp.tile([128, 512], F32, tag="ps")
            for n in range(NCH):
                sl = slice(512 * n, 512 * (n + 1))
                nc.vector.tensor_mul(aa[:, sl], tanh_a[:, sl], G_ot[:, sl])
                lastview = aa[:, sl].rearrange("p (m s) -> p m s", s=4)[:, :, 3]
                nc.vector.tensor_copy(aal16[:, 128 * n : 128 * (n + 1)], lastview)
                transpose128(aal16, saalT, n)
                nc.tensor.matmul(
                    psm,
                    lhsT=saalT[:, n, :],
                    rhs=sAst[:, n, :],
                    start=(n == 0),
                    stop=(n == NCH - 1),
                )
                # per-chunk partial s-sum: s012 chunk = aa[:,:,0]+aa[:,:,1]+aa[:,:,2]
                msl = slice(128 * n, 128 * (n + 1))
                aav_n = aa[:, sl].rearrange("p (m s) -> p m s", s=4)
                s01 = work.tile([128, 128], F32, tag="s01")
                nc.vector.tensor_add(s01, aav_n[:, :, 0], aav_n[:, :, 1])
                nc.vector.tensor_add(s012[:, msl], s01, aav_n[:, :, 2])

            # ---- m_new = s-sums + aa_last @ A_st ----
            m_new = singles.tile([128, 512], F32, tag="m_new")
            nc.vector.tensor_add(m_new, s012, psm)
            nc.gpsimd.dma_start(out=m_out, in_=m_new)

    nc.compile()
    return nc


def _get_nc():
    if "nc" not in _CACHE:
        _CACHE["nc"] = _build()
    return _CACHE["nc"]


def _chunked_T(x, nchunks):
    """[rows, cols] -> [128, nchunks*cols] with out[p, c*cols:...] = x[c*128+p, :]."""
    rows, cols = x.shape
    assert rows == nchunks * 128
    return np.ascontiguousarray(
        x.reshape(nchunks, 128, cols).transpose(1, 0, 2)
    ).reshape(128, nchunks * cols)


def _prep_inputs(inputs):
    f16 = np.float16
    f32 = np.float32
    x_t = np.asarray(inputs["x_t"], f32)
    m_t = np.asarray(inputs["m_t"], f32)
    a_t = np.asarray(inputs["a_t"], f32)

    # masks (idempotent with how setup_inputs builds the weights)
    eye = np.eye(M, dtype=f32)
    diag_mask = np.broadcast_to((1.0 - eye)[:, :, None], (M, M, D)).reshape(M, MD)
    A_th = np.asarray(inputs["A_th"], f32) * diag_mask
    A_ot = np.asarray(inputs["A_ot"], f32) * diag_mask
    tri = (np.arange(D)[:, None] < np.arange(D)[None, :]).astype(f32)
    pt_mask = (eye[:, None, :, None] * tri[None, :, None, :]).reshape(MD, MD)
    A_pt = np.asarray(inputs["A_pt"], f32) * pt_mask

    Am = {
        "fg": np.asarray(inputs["A_fg"], f32),
        "in": np.asarray(inputs["A_in"], f32),
        "th": A_th,
        "ot": A_ot,
    }
    shared = {}
    for g in ("fg", "in", "th", "ot"):
        # x-part first, then A-part (matches kernel chunk order)
        merged = np.concatenate([np.asarray(inputs[f"B_{g}"], f32), Am[g]], axis=0)
        shared[f"W_{g}"] = _chunked_T(merged.astype(f16), KW).reshape(128, KW, MD)
    shared["wball16"] = np.concatenate(
        [np.asarray(inputs[f"b_{g}"], f32).reshape(-1) for g in ("fg", "in", "th", "ot")]
        + [np.ones(128, f32)]
    ).astype(f16).reshape(1, -1)

    ast_c = _chunked_T(np.asarray(inputs["A_st"], f32).astype(f16), KM)
    aptd_c = _chunked_T(
        np.concatenate(
            [A_pt[128 * c : 128 * (c + 1), 128 * c : 128 * (c + 1)] for c in range(16)],
            axis=0,
        ).astype(f16),
        16,
    )
    ident = np.eye(128, dtype=f16)

    in_maps = []
    for i in range(NCORES):
        sl = slice(BL * i, BL * (i + 1))
        xs, ms, as_ = x_t[sl], m_t[sl], a_t[sl]
        im = dict(shared)
        mxT = np.concatenate(
            [np.ascontiguousarray(xs.T), np.ascontiguousarray(ms.T)], axis=0
        ).astype(f16)
        im["mxT16"] = _chunked_T(mxT, KW).reshape(128, KW, 128)
        alast_c = _chunked_T(np.ascontiguousarray(as_[:, 3::4].T).astype(f16), KM)
        at_c = _chunked_T(np.ascontiguousarray(as_.T).astype(f16), 16)
        im["stat_a1_16"] = alast_c
        im["stat_a2_16"] = np.concatenate(
            [at_c, aptd_c, as_.astype(f16), ident], axis=1
        )
        im["stat_b16"] = ast_c
        in_maps.append(im)
    return in_maps


def kernel(**inputs):
    global LAST_RESULT
    import os

    nc = _get_nc()
    in_maps = _prep_inputs(inputs)
    try:
        res = run_bass_kernel_spmd(nc, in_maps, list(range(NCORES)))
    except ModuleNotFoundError:
        # BASS_TRACE set but the NTFF profiling hook module is unavailable —
        # retry with tracing hard-disabled.
        os.environ["BASS_NEVER_TRACE"] = "1"
        res = run_bass_kernel_spmd(nc, in_maps, list(range(NCORES)))
    except Exception:
        # transient NRT device wedge (observed ~2/30 launches:
        # NRT_EXEC_UNIT_UNRECOVERABLE) — one retry usually succeeds
        res = run_bass_kernel_spmd(nc, in_maps, list(range(NCORES)))
    LAST_RESULT = res
    m_new = np.concatenate([res.results[i]["m_new_out"] for i in range(NCORES)], axis=0)
    a_new = np.concatenate([res.results[i]["a_new_out"] for i in range(NCORES)], axis=0)
    return (m_new, a_new)


# revision 26
# speedup vs baseline: 1.0162x; 1.0162x over previous
"""EDRN cell kernel for Trainium2, data-parallel over batch across 8 NeuronCores.

Strategy:
  - Shard batch B=1024 into 8 slices of 128 rows; replicate weights (fp16).
  - Mapping: batch rows on PSUM partitions, gate columns on the free dim.
    Stationary operands are transposed activation slices (host-prepped fp16,
    laid out [128, c, 128] so DMAs are contiguous), moving operands are weight
    row-chunks (fp16), fp32 PSUM accumulation.
  - Per gate the B(x)-part and A-part weights are merged host-side into one
    [128, 6, 2048] array with the x-part first, streamed as 3 ascending-size
    DMAs; biases are K=1 matmuls against a ones-row stationary.
  - fg/in run k-outer (matmuls chase the weight stream); th/ot run n-outer so
    each 512-column chunk finishes (incl. activation) early and the a_new/aa
    elementwise phases pipeline chunk-by-chunk with the remaining matmuls.
  - ot's x-part runs as a separate early accumulation phase overlapping the
    a_new elementwise work; its a_new_last-part joins per column chunk.
  - A_pt (block-diagonal strictly-upper 4x4) -> 16 [128,128] diag-block
    matmuls; a_new_last / aa_last transposed on the PE (transpose mode).
Elementwise math is fp32; matmul operands and the streamed copy of `a` are
fp16 (measured end-to-end error ~4e-4 relative).
"""

import numpy as np

import concourse.bass as bass
import concourse.mybir as mybir
import concourse.tile as tile
from concourse import bacc
from concourse.bass_utils import run_bass_kernel_spmd

N, M, D = 256, 512, 4
MD = M * D  # 2048
B = 1024
NCORES = 8
BL = B // NCORES  # 128 batch rows per core

F16 = mybir.dt.float16
F32 = mybir.dt.float32

KM = M // 128    # 4 K-chunks of the m/a_last contraction
KX = N // 128    # 2 K-chunks of the x contraction
KW = KM + KX     # 6 merged weight K-chunks per gate (x-part first)
NCH = MD // 512  # 4 column chunks of 512
SEGS = [(k, 1) for k in range(6)]  # single-chunk weight DMAs: pace matmuls per 512KB

# stat_a1: a_last^T chunks (tiny, lands between th's x and A weight chunks)
SA1_F = 512
# stat_a2 blob: a^T | A_pt diag blocks | a (natural fp16) | identity
SA_AT = 0
SA_APT = 2048
SA_A16 = 4096
SA_ID = 6144
SA2_F = SA_ID + 128  # 6272
# stat_b blob: A_st chunks only (needed last, keeps the ring tail short)
SB_F = 2048

_CACHE = {}
LAST_RESULT = None  # BassKernelResults of the most recent run (for test harness)


def _build():
    nc = bacc.Bacc(
        "TRN2", target_bir_lowering=False, debug=False, num_devices=NCORES
    )

    def din(name, shape, dt):
        return nc.dram_tensor(name, shape, dt, kind="ExternalInput").ap()

    def dout(name, shape, dt):
        return nc.dram_tensor(name, shape, dt, kind="ExternalOutput").ap()

    mxT = din("mxT16", [128, KW, 128], F16)   # x^T chunks 0..1, m^T chunks 2..5
    stat_a1 = din("stat_a1_16", [128, SA1_F], F16)
    stat_a2 = din("stat_a2_16", [128, SA2_F], F16)
    stat_b = din("stat_b16", [128, SB_F], F16)
    wball = din("wball16", [1, 4 * MD + 128], F16)  # 4 bias rows + ones row
    Wg = {g: din(f"W_{g}", [128, KW, MD], F16) for g in ("fg", "in", "th", "ot")}
    m_out = dout("m_new_out", [BL, M], F32)
    a_out = dout("a_new_out", [BL, MD], F32)

    GIDX = {"fg": 0, "in": 1, "th": 2, "ot": 3}
    AF = mybir.ActivationFunctionType

    with tile.TileContext(nc) as tc:
        with (
            tc.tile_pool(name="singles", bufs=1) as singles,
            tc.tile_pool(name="wpool", bufs=6) as wpool,
            tc.tile_pool(name="psum", bufs=8, space="PSUM") as pp,
            tc.tile_pool(name="work", bufs=3) as work,
        ):
            # sacrificial DMA: absorb the cold-DGE ramp before real weights
            dummy = singles.tile([128, 256], F16, tag="dummy")
            nc.sync.dma_start(out=dummy, in_=Wg["fg"][:, 0:1, 0:256])
            # mxT on the (otherwise idle) gpsimd ring — the scalar ring is
            # blocked ~1.3us by the ACT table load; biases stay on scalar
            smxT = singles.tile([128, KW, 128], F16, tag="smxT")
            nc.gpsimd.dma_start(out=smxT, in_=mxT)
            swb = singles.tile([1, 4 * MD + 128], F16, tag="swb")
            nc.scalar.dma_start(out=swb, in_=wball)
            sones = swb[:, 4 * MD : 4 * MD + 128]

            # PE warmup: dummy matmuls during the initial DMA prefix keep the
            # HAM activity window busy so the real stream starts at 2.4GHz
            # instead of the throttled 1.2GHz default.
            wu = singles.tile([128, 256], F16, tag="wu")
            nc.vector.memset(wu, 0.0)
            wups = pp.tile([64, 256], F32, tag="ps", name="wups")
            for _ in range(18):
                nc.tensor.matmul(
                    wups, lhsT=wu[:, 0:64], rhs=wu, start=True, stop=True
                )

            def lhs_for(statA, k):
                if k < KX or statA is None:
                    return smxT[:, k, :]
                return statA[:, k - KX, :]

            def gate_load(gname):
                tiles = []
                for start_c, nch in SEGS:
                    w = wpool.tile(
                        [128, nch, MD], F16, tag=f"w{nch}", name=f"w_{gname}_{start_c}"
                    )
                    nc.sync.dma_start(
                        out=w, in_=Wg[gname][:, start_c : start_c + nch, :]
                    )
                    for kk in range(nch):
                        tiles.append((w, kk))
                return tiles

            def bias_mm(gname, psums, n, stop=True):
                boff = GIDX[gname] * MD
                nc.tensor.matmul(
                    psums[n],
                    lhsT=sones,
                    rhs=swb[:, boff + 512 * n : boff + 512 * (n + 1)],
                    start=False,
                    stop=stop,
                )

            def new_psums(gname):
                return [
                    pp.tile([128, 512], F32, tag="ps", name=f"ps_{gname}_{n}")
                    for n in range(NCH)
                ]

            def gate_kcontig(gname, func):
                """k-outer: matmuls chase the weight stream (fg/in)."""
                G = singles.tile([128, MD], F32, tag=f"G_{gname}")
                psums = new_psums(gname)
                for k, (w, kk) in enumerate(gate_load(gname)):
                    lhsT = lhs_for(None, k)
                    for n in range(NCH):
                        nc.tensor.matmul(
                            psums[n],
                            lhsT=lhsT,
                            rhs=w[:, kk, 512 * n : 512 * (n + 1)],
                            start=(k == 0),
                            stop=False,
                        )
                for n in range(NCH):
                    bias_mm(gname, psums, n)
                    nc.scalar.activation(
                        out=G[:, 512 * n : 512 * (n + 1)], in_=psums[n], func=func
                    )
                return G

            G_fg = gate_kcontig("fg", AF.Sigmoid)

            # stationaries for th / a_new, loaded while fg/in stream
            ssa = singles.tile([128, SA_F], F16, tag="ssa")
            nc.gpsimd.dma_start(out=ssa, in_=stat_a)
            salastT = ssa[:, SA_ALAST : SA_ALAST + 512].rearrange(
                "p (c k) -> p c k", k=128
            )
            saT = ssa[:, SA_AT : SA_AT + 2048].rearrange("p (c k) -> p c k", k=128)
            sAptd = ssa[:, SA_APT : SA_APT + 2048].rearrange(
                "p (c k) -> p c k", k=128
            )
            sa16 = ssa[:, SA_A16 : SA_A16 + 2048]
            sident = ssa[:, SA_ID : SA_ID + 128]

            G_in = gate_kcontig("in", AF.Sigmoid)

            ssb = singles.tile([128, SB_F], F16, tag="ssb")
            nc.gpsimd.dma_start(out=ssb, in_=stat_b)
            sAst = ssb[:, SB_AST : SB_AST + 2048].rearrange(
                "p (c m) -> p c m", m=512
            )
            sa16 = ssb[:, SB_A16 : SB_A16 + 2048]
            sident = ssb[:, SB_ID : SB_ID + 128]

            def transpose128(src16, dst, n):
                """dst[:, n, :] = src16[:, 128n:128(n+1)].T via PE transpose."""
                pt = pp.tile([128, 128], F16, tag="ps", name=f"pt_{dst.name}_{n}")
                nc.tensor.transpose(
                    pt, src16[:, 128 * n : 128 * (n + 1)], sident
                )
                nc.vector.tensor_copy(dst[:, n, :], pt)

            # ---- th gate (k-outer), then a_new phase ----
            # ring order: th x-chunks, a_last^T (tiny), th A-chunks, then the
            # bulky a^T/A_pt/a/ident blob draining behind th's matmuls
            th_tiles = []

            def _load_th(lo, hi):
                for c in range(lo, hi):
                    w = wpool.tile([128, 1, MD], F16, tag="w1", name=f"w_th_{c}")
                    nc.sync.dma_start(out=w, in_=Wg["th"][:, c : c + 1, :])
                    th_tiles.append((w, 0))

            _load_th(0, KX)
            ssa1 = singles.tile([128, SA1_F], F16, tag="ssa1")
            nc.sync.dma_start(out=ssa1, in_=stat_a1)
            salastT = ssa1.rearrange("p (c k) -> p c k", k=128)
            _load_th(KX, KW)
            ssa2 = singles.tile([128, SA2_F], F16, tag="ssa2")
            nc.sync.dma_start(out=ssa2, in_=stat_a2)
            saT = ssa2[:, SA_AT : SA_AT + 2048].rearrange("p (c k) -> p c k", k=128)
            sAptd = ssa2[:, SA_APT : SA_APT + 2048].rearrange(
                "p (c k) -> p c k", k=128
            )
            sa16 = ssa2[:, SA_A16 : SA_A16 + 2048]
            sident = ssa2[:, SA_ID : SA_ID + 128]

            G_th = singles.tile([128, MD], F32, tag="G_th")
            th_psums = new_psums("th")
            for k, (w, kk) in enumerate(th_tiles):
                lhsT = lhs_for(salastT, k)
                for n in range(NCH):
                    nc.tensor.matmul(
                        th_psums[n],
                        lhsT=lhsT,
                        rhs=w[:, kk, 512 * n : 512 * (n + 1)],
                        start=(k == 0),
                        stop=False,
                    )
            for n in range(NCH):
                for s in range(4):
                    c = 4 * n + s
                    nc.tensor.matmul(
                        th_psums[n][:, 128 * s : 128 * (s + 1)],
                        lhsT=saT[:, c, :],
                        rhs=sAptd[:, c, :],
                        start=False,
                        stop=False,
                        skip_group_check=True,
                    )
            for n in range(NCH):
                bias_mm("th", th_psums, n)
                nc.scalar.activation(
                    out=G_th[:, 512 * n : 512 * (n + 1)],
                    in_=th_psums[n],
                    func=AF.Tanh,
                )

            # ---- ot x-part first: independent of a_new, fills the PE while
            #      the a_new elementwise phase runs ----
            G_ot = singles.tile([128, MD], F32, tag="G_ot")
            ot_psums = new_psums("ot")
            ot_tiles = gate_load("ot")
            ssb = singles.tile([128, SB_F], F16, tag="ssb")
            nc.sync.dma_start(out=ssb, in_=stat_b)
            sAst = ssb.rearrange("p (c m) -> p c m", m=512)
            for k, (w, kk) in enumerate(ot_tiles[:KX]):
                for n in range(NCH):
                    nc.tensor.matmul(
                        ot_psums[n],
                        lhsT=smxT[:, k, :],
                        rhs=w[:, kk, 512 * n : 512 * (n + 1)],
                        start=(k == 0),
                        stop=False,
                    )

            # ---- a_new = a * G_fg + G_th * G_in, plus last-slice transpose ----
            a_new = singles.tile([128, MD], F32, tag="a_new")
            anl16 = singles.tile([128, 512], F16, tag="anl16")
            sanlT = singles.tile([128, KM, 128], F16, tag="sanlT")
            for n in range(NCH):
                sl = slice(512 * n, 512 * (n + 1))
                t1 = work.tile([128, 512], F32, tag="t1")
                nc.vector.tensor_mul(t1, G_th[:, sl], G_in[:, sl])
                t2 = work.tile([128, 512], F32, tag="t2")
                nc.gpsimd.tensor_mul(t2, sa16[:, sl], G_fg[:, sl])
                nc.vector.tensor_add(a_new[:, sl], t1, t2)
                lastview = a_new[:, sl].rearrange("p (m s) -> p m s", s=4)[:, :, 3]
                nc.vector.tensor_copy(anl16[:, 128 * n : 128 * (n + 1)], lastview)
                transpose128(anl16, sanlT, n)
            nc.gpsimd.dma_start(out=a_out, in_=a_new)

            # ---- ot a_new_last-part ----
            for k, (w, kk) in list(enumerate(ot_tiles))[KX:]:
                for n in range(NCH):
                    nc.tensor.matmul(
                        ot_psums[n],
                        lhsT=sanlT[:, k - KX, :],
                        rhs=w[:, kk, 512 * n : 512 * (n + 1)],
                        start=False,
                        stop=False,
                    )
            # tanh(a_new) is independent of G_ot — compute it during the ot
            # matmul phase while the scalar queue is otherwise idle
            tanh_a = singles.tile([128, MD], F32, tag="tanh_a")
            for n in range(NCH):
                sl = slice(512 * n, 512 * (n + 1))
                nc.scalar.activation(
                    out=tanh_a[:, sl], in_=a_new[:, sl], func=AF.Tanh
                )

            for n in range(NCH):
                bias_mm("ot", ot_psums, n)
                nc.scalar.activation(
                    out=G_ot[:, 512 * n : 512 * (n + 1)],
                    in_=ot_psums[n],
                    func=AF.Sigmoid,
                )

            # ---- aa = tanh(a_new) * G_ot; m_new accumulates as chunks finish ----
            aa = singles.tile([128, MD], F32, tag="aa")
            aal16 = singles.tile([128, 512], F16, tag="aal16")
            saalT = singles.tile([128, KM, 128], F16, tag="saalT")
            s012 = singles.tile([128, 512], F32, tag="s012")
            psm = pp.tile([128, 512], F32, tag="ps")
            for n in range(NCH):
                sl = slice(512 * n, 512 * (n + 1))
                nc.vector.tensor_mul(aa[:, sl], tanh_a[:, sl], G_ot[:, sl])
                lastview = aa[:, sl].rearrange("p (m s) -> p m s", s=4)[:, :, 3]
                nc.vector.tensor_copy(aal16[:, 128 * n : 128 * (n + 1)], lastview)
                transpose128(aal16, saalT, n)
                nc.tensor.matmul(
                    psm,
                    lhsT=saalT[:, n, :],
                    rhs=sAst[:, n, :],
                    start=(n == 0),
                    stop=(n == NCH - 1),
                )
                # per-chunk partial s-sum: s012 chunk = aa[:,:,0]+aa[:,:,1]+aa[:,:,2]
                msl = slice(128 * n, 128 * (n + 1))
                aav_n = aa[:, sl].rearrange("p (m s) -> p m s", s=4)
                s01 = work.tile([128, 128], F32, tag="s01")
                nc.vector.tensor_add(s01, aav_n[:, :, 0], aav_n[:, :, 1])
                nc.vector.tensor_add(s012[:, msl], s01, aav_n[:, :, 2])

            # ---- m_new = s-sums + aa_last @ A_st ----
            m_new = singles.tile([128, 512], F32, tag="m_new")
            nc.vector.tensor_add(m_new, s012, psm)
            nc.gpsimd.dma_start(out=m_out, in_=m_new)

    nc.compile()
    return nc


def _get_nc():
    if "nc" not in _CACHE:
        _CACHE["nc"] = _build()
    return _CACHE["nc"]


def _chunked_T(x, nchunks):
    """[rows, cols] -> [128, nchunks*cols] with out[p, c*cols:...] = x[c*128+p, :]."""
    rows, cols = x.shape
    assert rows == nchunks * 128
    return np.ascontiguousarray(
        x.reshape(nchunks, 128, cols).transpose(1, 0, 2)
    ).reshape(128, nchunks * cols)


def _prep_inputs(inputs):
    f16 = np.float16
    f32 = np.float32
    x_t = np.asarray(inputs["x_t"], f32)
    m_t = np.asarray(inputs["m_t"], f32)
    a_t = np.asarray(inputs["a_t"], f32)

    # masks (idempotent with how setup_inputs builds the weights)
    eye = np.eye(M, dtype=f32)
    diag_mask = np.broadcast_to((1.0 - eye)[:, :, None], (M, M, D)).reshape(M, MD)
    A_th = np.asarray(inputs["A_th"], f32) * diag_mask
    A_ot = np.asarray(inputs["A_ot"], f32) * diag_mask
    tri = (np.arange(D)[:, None] < np.arange(D)[None, :]).astype(f32)
    pt_mask = (eye[:, None, :, None] * tri[None, :, None, :]).reshape(MD, MD)
    A_pt = np.asarray(inputs["A_pt"], f32) * pt_mask

    Am = {
        "fg": np.asarray(inputs["A_fg"], f32),
        "in": np.asarray(inputs["A_in"], f32),
        "th": A_th,
        "ot": A_ot,
    }
    shared = {}
    for g in ("fg", "in", "th", "ot"):
        # x-part first, then A-part (matches kernel chunk order)
        merged = np.concatenate([np.asarray(inputs[f"B_{g}"], f32), Am[g]], axis=0)
        shared[f"W_{g}"] = _chunked_T(merged.astype(f16), KW).reshape(128, KW, MD)
    shared["wball16"] = np.concatenate(
        [np.asarray(inputs[f"b_{g}"], f32).reshape(-1) for g in ("fg", "in", "th", "ot")]
        + [np.ones(128, f32)]
    ).astype(f16).reshape(1, -1)

    ast_c = _chunked_T(np.asarray(inputs["A_st"], f32).astype(f16), KM)
    aptd_c = _chunked_T(
        np.concatenate(
            [A_pt[128 * c : 128 * (c + 1), 128 * c : 128 * (c + 1)] for c in range(16)],
            axis=0,
        ).astype(f16),
        16,
    )
    ident = np.eye(128, dtype=f16)

    in_maps = []
    for i in range(NCORES):
        sl = slice(BL * i, BL * (i + 1))
        xs, ms, as_ = x_t[sl], m_t[sl], a_t[sl]
        im = dict(shared)
        mxT = np.concatenate(
            [np.ascontiguousarray(xs.T), np.ascontiguousarray(ms.T)], axis=0
        ).astype(f16)
        im["mxT16"] = _chunked_T(mxT, KW).reshape(128, KW, 128)
        alast_c = _chunked_T(np.ascontiguousarray(as_[:, 3::4].T).astype(f16), KM)
        at_c = _chunked_T(np.ascontiguousarray(as_.T).astype(f16), 16)
        im["stat_a1_16"] = alast_c
        im["stat_a2_16"] = np.concatenate(
            [at_c, aptd_c, as_.astype(f16), ident], axis=1
        )
        im["stat_b16"] = ast_c
        in_maps.append(im)
    return in_maps


def kernel(**inputs):
    global LAST_RESULT
    import os

    nc = _get_nc()
    in_maps = _prep_inputs(inputs)
    try:
        res = run_bass_kernel_spmd(nc, in_maps, list(range(NCORES)))
    except ModuleNotFoundError:
        # BASS_TRACE set but the NTFF profiling hook module is unavailable —
        # retry with tracing hard-disabled.
        os.environ["BASS_NEVER_TRACE"] = "1"
        res = run_bass_kernel_spmd(nc, in_maps, list(range(NCORES)))
    except Exception:
        # transient NRT device wedge (observed ~2/30 launches:
        # NRT_EXEC_UNIT_UNRECOVERABLE) — one retry usually succeeds
        res = run_bass_kernel_spmd(nc, in_maps, list(range(NCORES)))
    LAST_RESULT = res
    m_new = np.concatenate([res.results[i]["m_new_out"] for i in range(NCORES)], axis=0)
    a_new = np.concatenate([res.results[i]["a_new_out"] for i in range(NCORES)], axis=0)
    return (m_new, a_new)
